# revision 63
# baseline (speedup 1.0000x reference)
"""MHSA3D Trainium2 kernel: 8-way head-parallel flash-style attention.

Problem (hardcoded): B=1, C=128, D=H=W=16 -> N=4096 tokens, 8 heads,
dh=16, dv=128.  Each of the 8 NeuronCores computes one head end-to-end:
qkv projection (its head's slice), S^T = k'^T q' logits in [j, i]
layout, exp, PV accumulation with an appended ones-row producing the
softmax denominator, then normalize.

Perf-critical choices:
- The qk contraction is only dh=16, but K=16 matmuls never trip the PE
  HAM activity monitor, so the PE clock stays throttled at 1.2 GHz.
  We zero-pad the contraction to K=96 (stationary k' rows 16-95 are
  exact zeros); matmul time scales with N only, and K=96 matmuls keep
  the PE warm at 2.4 GHz.
- q'/k' in fp16 (logits reach +-48, but q/k values are small; fp16's
  extra mantissa bits halve the logit rounding error vs bf16), P and v
  in bf16 (P reaches exp(42), beyond fp16 range).
- exp is the ScalarE bottleneck (one column/cycle at 1.2 GHz = 109 us
  per head if ScalarE does all of it, while the PE floor is ~111 us).
  5 of every 16 j-block-pair groups instead compute exp on the Vector
  engine with a Schraudolph bit-trick: bf16(exp(x)) ~= bitcast_bf16(
  int16(x * 128/ln2 + 16256 + CB)), one tensor_scalar instruction.
  The softmax self-normalizes, so the trunc-vs-round conversion mode
  (a constant shift of CB => a constant scale on P) cancels exactly;
  CB centers the linear-mantissa sawtooth so Schraudolph blocks carry
  no systematic weight bias vs ScalarE blocks.
- v-bias is folded into the v rows (out = sum P (v + bv) / sum P ==
  softmax @ v + bv), so there is no post-normalize add.
- The normalize reciprocal is a bit-trick seed (bits(1/d) ~= K -
  bits(d)) plus one Newton step, as three ~0.6us DVE ops emitted one
  group apart -- DVE's RECIPROCAL instruction is a single 3.3us
  queue-blocker that stalls Schraudolph exps queued behind it, blowing
  ~15us of pipeline; reciprocal_approx_fast diverges on our e^0..e^42
  denominators (bf16 r needs f32 exponent range: 1/denom underflows
  fp16).  ScalarE copies the numerator out of PSUM in parallel; the
  idle GpSimd engine broadcasts 1/denom across the 16 output rows.
- Startup: w dispatches first (it gates the projection), then per
  1024-column round the two x halves go to the sync+scalar DMA queues
  in parallel while gpsimd moves that round's fp16 bias quarter, so
  chunk arrival tracks the first eighth's consumption order.
  Zero-fills are engine memsets, not DMA fan-outs, keeping the GpSimd
  queue free for the k' partition-shift copies that gate the first qk.
- PV for group g is emitted after qk for group g+2 (g+4 for Vector-exp
  groups) so the PE FIFO never head-of-line blocks on the exp; the
  per-eighth normalize tail is deferred into the next eighth the same
  way.

Host side: fold the 1/sqrt(dh) scale into wq/bq, fold (b_k + positional
embedding) into a [16, N] bias plane, slice per-head weights, run the
SPMD program on cores 0-7, and concatenate the per-head [16, N] outputs.
"""

import math

import numpy as np

NHEADS = 8
DV = 128
DH = DV // NHEADS  # 16
C = 128
N = 4096
ECOLS = 512        # i-columns handled per output tile ("eighth")
NE = N // ECOLS    # 8
JW = 128           # keys per j-block
NJB = N // JW      # 32
NG = NJB // 2      # 16 j-block pairs per eighth

# exp engine per group within an eighth: 'A' ScalarE table exp,
# 'D' VectorE Schraudolph.  gi 15 must be 'A': its defer-2 stop-PV has
# to be emitted before the next eighth's gi-2 tail reads the
# denominator.
ENGMAP = ['A', 'D', 'A', 'A', 'D', 'A', 'A', 'D',
          'A', 'A', 'D', 'A', 'D', 'A', 'D', 'A']
DEFER = {'A': 2, 'D': 3}   # groups between qk and pv emission

# f32 reciprocal bit-trick: bits(1/d) ~= K - bits(d), then one
# Newton-Raphson step; max rel err 0.26% before the bf16 store.
K_RECIP = float(0x7EF30000)

A_SCHR = 128.0 / math.log(2.0)
B_SCHR = 16256.0 - 4.77

_compiled = None


def _build_program():
    import concourse.bacc as bacc
    import concourse.mybir as mybir
    import concourse.tile as tile

    f32 = mybir.dt.float32
    bf16 = mybir.dt.bfloat16
    fp16 = mybir.dt.float16
    i16 = mybir.dt.int16
    EXP = mybir.ActivationFunctionType.Exp
    LN = mybir.ActivationFunctionType.Ln
    ADD = mybir.AluOpType.add
    MULT = mybir.AluOpType.mult

    nc = bacc.Bacc("TRN2", target_bir_lowering=False, debug=False,
                   num_devices=NHEADS)

    x_d = nc.dram_tensor("x", [C, N], fp16, kind="ExternalInput")
    # w cols: 0-15 wq*scale, 16-31 wk, 32-47 wv
    w_d = nc.dram_tensor("w", [C, 48], fp16, kind="ExternalInput")
    # bias plane rows: 0-15 bq*scale (bcast), 16-31 bk+emb.  fp16: q'/k'
    # are rounded to fp16 right after the bias add anyway, so f32 bias
    # precision would be wasted; fp16 halves the startup DMA bytes.
    b_d = nc.dram_tensor("bias", [32, N], fp16, kind="ExternalInput")
    # bv tiled to [128, 32] so the v^T augment add can fold it in
    bvt_d = nc.dram_tensor("bvt", [128, 32], f32, kind="ExternalInput")
    o_d = nc.dram_tensor("out", [DH, N], f32, kind="ExternalOutput")

    with tile.TileContext(nc) as tc:
        with (
            tc.tile_pool(name="const", bufs=1) as const,
            tc.tile_pool(name="pt", bufs=8) as ptp,
            tc.tile_pool(name="o", bufs=2) as op,
            tc.tile_pool(name="st", bufs=3, space="PSUM") as stp,
            tc.tile_pool(name="acc", bufs=2, space="PSUM") as accp,
        ):
            x_s = const.tile([C, N], fp16)
            w_s = const.tile([C, 48], fp16)
            # bias in 4 column tiles so the projection add for chunk ch
            # only waits on its own quarter (tile-granular deps).
            biasf = [const.tile([32, 1024], fp16, name=f"biasf{q}")
                     for q in range(4)]
            bvt_s = const.tile([128, 32], f32)
            # K=96 contraction: the smallest K that keeps the PE HAM warm.
            # Split into per-512-column tiles so consumers only wait on
            # their own chunk (Tile tracks deps at whole-tile granularity).
            # qz rows: 0-15 q'; 16-31 k' (junk for the matmul); 32-95 zero
            qzt = [const.tile([96, 512], fp16, name=f"qzt{c}")
                   for c in range(8)]
            # kz rows: 0-15 k'; 16-95 exact zero (masks the qz junk rows)
            kzt = [const.tile([96, 512], fp16, name=f"kzt{c}")
                   for c in range(8)]
            vaugT = const.tile([128, 33 * NJB], bf16)  # [j, (v^T |0*16| 1)]
            va3 = vaugT[:].rearrange("p (c s) -> p c s", s=33)
            ones16 = const.tile([1, DH], bf16)
            zerob = const.tile([128, 1], f32)
            scratch1 = const.tile([128, 1], f32)

            # Startup DMA schedule, paced to the consumption order of the
            # first eighth (qk group 2c+t consumes kzt[2c+t] built from x
            # columns [1024c, 1024c+1024)): per round c, the two x halves
            # of chunk c go to sync+scalar in parallel while gpsimd moves
            # that round's bias quarter.  Engine-level constants (zerob
            # gates the first exp's bias) go on the otherwise-idle DVE.
            nc.vector.memset(zerob[:], 0.0)
            nc.vector.memset(ones16[:], 1.0)
            nc.sync.dma_start(w_s[:], w_d.ap())
            for q in range(4):
                qs = slice(q * 1024, (q + 1) * 1024)
                nc.sync.dma_start(x_s[0:64, qs], x_d.ap()[0:64, qs])
                nc.scalar.dma_start(x_s[64:128, qs], x_d.ap()[64:128, qs])
                nc.gpsimd.dma_start(biasf[q][:], b_d.ap()[:, qs])
            nc.gpsimd.dma_start(bvt_s[:], bvt_d.ap())
            nc.vector.memset(va3[:, :, 16:32], 0.0)
            nc.vector.memset(va3[:, :, 32:33], 1.0)
            # Warm the exp table set while DMAs run.
            nc.scalar.activation(scratch1[:], zerob[:], EXP, bias=zerob[:])

            # qkv projection: psum[0:32] = w[:, 0:32]^T @ x, + bias plane.
            # Zero-region memsets are interleaved per chunk so the first
            # chunks' adds/shifts aren't queued behind 30 memsets.
            for ch in range(8):
                cs = slice(ch * 512, (ch + 1) * 512)
                nc.vector.memset(qzt[ch][32:64, :], 0.0)
                nc.vector.memset(qzt[ch][64:96, :], 0.0)
                nc.gpsimd.memset(kzt[ch][32:64, :], 0.0)
                nc.gpsimd.memset(kzt[ch][64:96, :], 0.0)
                ps = stp.tile([32, 512], f32, tag="st")
                nc.tensor.matmul(ps[:], lhsT=w_s[:, 0:32], rhs=x_s[:, cs],
                                 start=True, stop=True)
                nc.vector.tensor_tensor(qzt[ch][0:32, :], ps[:],
                                        biasf[ch // 2][:, (ch % 2) * 512:
                                                       (ch % 2) * 512 + 512],
                                        ADD)
                # k' into the zero-padded stationary tensor (partition
                # shift; only a DMA can cross partitions).  Copying 32
                # rows brings 16 zeros from qzt's zeroed region along,
                # covering kzt rows 16:32 (engine memsets must start at a
                # 32-aligned partition, so they can't zero those rows).
                nc.gpsimd.dma_start(kzt[ch][0:2 * DH, :],
                                    qzt[ch][DH:3 * DH, :])

            # v^T psum staging; the projection MMs themselves are
            # interleaved into eighth 0 of the main loop.
            vps = stp.tile([128, 512], f32, tag="st")
            vp3 = vps[:].rearrange("p (c s) -> p c s", s=16)

            def emit_vt(jb):
                nc.tensor.matmul(vps[:, jb * 16:(jb + 1) * 16],
                                 lhsT=x_s[:, jb * JW:(jb + 1) * JW],
                                 rhs=w_s[:, 32:48],
                                 start=True, stop=True)

            bv3 = bvt_s[:].rearrange("p (c s) -> p c s", s=16)

            def emit_vt_aug(c0, c1):
                # v' = v + bv: the ones-row in vaug turns sum P bv into
                # bv * denominator, so normalize needs no post-add.
                nc.vector.tensor_tensor(va3[:, c0:c1, 0:16], vp3[:, c0:c1, :],
                                        bv3[:, 0:c1 - c0, :], ADD)

            def make_pv(pt, jbs, acc, start, stop):
                def emit():
                    for t, jb in enumerate(jbs):
                        nc.tensor.matmul(
                            acc[:],
                            lhsT=vaugT[:, 33 * jb:33 * jb + 33],
                            rhs=pt[:, 512 * t:512 * (t + 1)],
                            start=(start and t == 0),
                            stop=(stop and t == len(jbs) - 1))
                return emit

            def make_tail_a(acc):
                # 1/denom in three ~0.6us DVE ops emitted one group apart
                # (DVE's RECIPROCAL is a single 3.3us queue-blocking
                # instruction; reciprocal_approx_fast diverges on our
                # e^0..e^42 denominators; a ScalarE exp(-ln) churns
                # activation tables).  r holds (t-2)*r0 in bf16; the
                # GpSimd broadcast then fans (2-t)*r0 across rows.
                o16 = op.tile([DH, ECOLS], f32, tag="o16")
                r0 = op.tile([1, ECOLS], f32, tag="r0")
                tn = op.tile([1, ECOLS], f32, tag="tn")
                r = op.tile([1, ECOLS], bf16, tag="r")

                def emit_seed():
                    nc.vector.tensor_scalar(
                        r0[:].bitcast(mybir.dt.int32),
                        acc[32:33, :].bitcast(mybir.dt.int32),
                        -1.0, K_RECIP, MULT, ADD)

                def emit_o16():
                    # gi4 is a DVE-exp slot, so ScalarE is idle there:
                    # the copy no longer bumps an exp inside an A-run.
                    nc.scalar.copy(o16[:], acc[0:DH, :])

                def emit_nr_mul():
                    # tn = -denom*r0 so the finish computes (2-t)*r0
                    # without a separate negate.
                    nc.vector.scalar_tensor_tensor(
                        tn[:], acc[32:33, :], -1.0, r0[:], MULT, MULT)

                def emit_nr_fin():
                    # gi5 is an ACT-exp slot, so DVE is idle there: the
                    # finish no longer queues behind the gi4 exp.
                    nc.vector.scalar_tensor_tensor(
                        r[:], tn[:], 2.0, r0[:], ADD, MULT)
                return emit_seed, emit_nr_mul, emit_o16, emit_nr_fin, o16, r

            def make_tail_b(o16, r, es):
                def emit():
                    # Broadcast 1/denom across the 16 output rows on the
                    # idle GpSimd engine (frees a PE matmul + PSUM slot).
                    bc = op.tile([DH, ECOLS], bf16, tag="bc")
                    nc.gpsimd.partition_broadcast(bc[:], r[:], channels=DH)
                    ost = op.tile([DH, ECOLS], f32, tag="ost")
                    nc.vector.tensor_tensor(ost[:], o16[:], bc[:],
                                            MULT)
                    nc.sync.dma_start(o_d.ap()[:, es], ost[:])
                return emit

            from collections import deque
            pend = deque()  # (due_gidx, emit_fn)
            GROUPS = [(2 * g, 2 * g + 1) for g in range(NG)]
            pending_a = []
            pending_b = None
            for e in range(NE):
                es = slice(e * ECOLS, (e + 1) * ECOLS)
                acc = accp.tile([33, ECOLS], f32)
                for gi, jbs in enumerate(GROUPS):
                    gidx = e * NG + gi
                    if e == 0:
                        for jb in jbs:
                            emit_vt(jb)
                        emit_vt_aug(jbs[0], jbs[-1] + 1)
                    fw = 512 * len(jbs)
                    st = stp.tile([128, 1024], f32, tag="st")
                    for t, jb in enumerate(jbs):
                        kc = kzt[jb // 4][:, (jb % 4) * JW:(jb % 4 + 1) * JW]
                        nc.tensor.matmul(st[:, 512 * t:512 * (t + 1)],
                                         lhsT=kc, rhs=qzt[e][:],
                                         start=True, stop=True)
                    pt = ptp.tile([128, 1024], bf16)
                    eng = ENGMAP[gi]
                    if eng == 'A':
                        nc.scalar.activation(pt[:, 0:fw], st[:, 0:fw], EXP,
                                             bias=zerob[:])
                    else:
                        nc.vector.tensor_scalar(
                            pt[:, 0:fw].bitcast(i16), st[:, 0:fw],
                            A_SCHR, B_SCHR, MULT, ADD)
                    while pend and pend[0][0] <= gidx:
                        pend.popleft()[1]()
                    if pending_a and gi in (2, 3, 4, 5):
                        pending_a.pop(0)()
                    if pending_b is not None and gi == 6:
                        pending_b()
                        pending_b = None
                    pend.append((gidx + DEFER[eng],
                                 make_pv(pt, jbs, acc,
                                         start=(gi == 0),
                                         stop=(gi == len(GROUPS) - 1))))
                e_seed, e_mul, e_o16, e_fin, o16, r = make_tail_a(acc)
                pending_a = [e_seed, e_mul, e_o16, e_fin]
                if pending_b is not None:
                    pending_b()
                pending_b = make_tail_b(o16, r, es)
            while pend:
                pend.popleft()[1]()
            for f in pending_a:
                f()
            pending_b()

    nc.compile()
    return nc


def _get_program():
    global _compiled
    if _compiled is None:
        _compiled = _build_program()
    return _compiled


def _prepare_core_inputs(x, w_qkv, b_qkv, emb_d, emb_h, emb_w):
    x2 = np.ascontiguousarray(
        np.asarray(x, np.float32).reshape(C, N)).astype(np.float16)
    w_qkv = np.asarray(w_qkv, np.float32)
    b_qkv = np.asarray(b_qkv, np.float32)
    scale = DH ** -0.5
    emb = (np.asarray(emb_d, np.float32)
           + np.asarray(emb_h, np.float32)
           + np.asarray(emb_w, np.float32)).reshape(DH, N)
    in_maps = []
    for h in range(NHEADS):
        qc = slice(h * DH, (h + 1) * DH)
        kc = slice(DV + h * DH, DV + (h + 1) * DH)
        vc = slice(2 * DV + h * DH, 2 * DV + (h + 1) * DH)
        w = np.empty((C, 48), np.float32)
        w[:, 0:16] = w_qkv[:, qc] * scale
        w[:, 16:32] = w_qkv[:, kc]
        w[:, 32:48] = w_qkv[:, vc]
        w = w.astype(np.float16)
        bias = np.empty((32, N), np.float32)
        bias[0:16, :] = (b_qkv[qc] * scale)[:, None]
        bias[16:32, :] = b_qkv[kc][:, None] + emb
        bias = bias.astype(np.float16)
        bvt = np.ascontiguousarray(
            np.tile(b_qkv[vc].astype(np.float32), (128, 2)))
        in_maps.append({"x": x2, "w": w, "bias": bias, "bvt": bvt})
    return in_maps


def kernel(x, w_qkv, b_qkv, emb_d, emb_h, emb_w):
    from concourse.bass_utils import run_bass_kernel_spmd

    nc = _get_program()
    in_maps = _prepare_core_inputs(x, w_qkv, b_qkv, emb_d, emb_h, emb_w)
    res = run_bass_kernel_spmd(nc, in_maps, list(range(NHEADS)))
    out = np.empty((DV, N), np.float32)
    for h in range(NHEADS):
        out[h * DH:(h + 1) * DH, :] = res.results[h]["out"]
    return out.reshape(1, DV, 16, 16, 16)


# revision 64
# speedup vs baseline: 1.2207x; 1.2207x over previous
"""MHSA3D Trainium2 kernel: 8-way head-parallel flash-style attention.

Problem (hardcoded): B=1, C=128, D=H=W=16 -> N=4096 tokens, 8 heads,
dh=16, dv=128.  Each of the 8 NeuronCores computes one head end-to-end:
qkv projection (its head's slice), S^T = k'^T q' logits in [j, i]
layout, exp, PV accumulation with an appended ones-row producing the
softmax denominator, then normalize.

Perf-critical choices:
- The qk contraction is only dh=16, but K=16 matmuls never trip the PE
  HAM activity monitor, so the PE clock stays throttled at 1.2 GHz.
  We zero-pad the contraction to K=96 (stationary k' rows 16-95 are
  exact zeros); matmul time scales with N only, and K=96 matmuls keep
  the PE warm at 2.4 GHz.
- q'/k' in fp16 (logits reach +-48, but q/k values are small; fp16's
  extra mantissa bits halve the logit rounding error vs bf16), P and v
  in bf16 (P reaches exp(42), beyond fp16 range).
- exp is the ScalarE bottleneck (one column/cycle at 1.2 GHz = 109 us
  per head if ScalarE does all of it, while the PE floor is ~111 us).
  5 of every 16 j-block-pair groups instead compute exp on the Vector
  engine with a Schraudolph bit-trick: bf16(exp(x)) ~= bitcast_bf16(
  int16(x * 128/ln2 + 16256 + CB)), one tensor_scalar instruction.
  The softmax self-normalizes, so the trunc-vs-round conversion mode
  (a constant shift of CB => a constant scale on P) cancels exactly;
  CB centers the linear-mantissa sawtooth so Schraudolph blocks carry
  no systematic weight bias vs ScalarE blocks.
- v-bias is folded into the v rows (out = sum P (v + bv) / sum P ==
  softmax @ v + bv), so there is no post-normalize add.
- The normalize reciprocal is a bit-trick seed (bits(1/d) ~= K -
  bits(d)) plus one Newton step, as three ~0.6us DVE ops emitted one
  group apart -- DVE's RECIPROCAL instruction is a single 3.3us
  queue-blocker that stalls Schraudolph exps queued behind it, blowing
  ~15us of pipeline; reciprocal_approx_fast diverges on our e^0..e^42
  denominators (bf16 r needs f32 exponent range: 1/denom underflows
  fp16).  ScalarE copies the numerator out of PSUM in parallel; the
  idle GpSimd engine broadcasts 1/denom across the 16 output rows.
- Startup: w dispatches first (it gates the projection), then per
  1024-column round the two x halves go to the sync+scalar DMA queues
  in parallel while gpsimd moves that round's fp16 bias quarter, so
  chunk arrival tracks the first eighth's consumption order.
  Zero-fills are engine memsets, not DMA fan-outs, keeping the GpSimd
  queue free for the k' partition-shift copies that gate the first qk.
- PV for group g is emitted after qk for group g+2 (g+4 for Vector-exp
  groups) so the PE FIFO never head-of-line blocks on the exp; the
  per-eighth normalize tail is deferred into the next eighth the same
  way.

Host side: fold the 1/sqrt(dh) scale into wq/bq, fold (b_k + positional
embedding) into a [16, N] bias plane, slice per-head weights, run the
SPMD program on cores 0-7, and concatenate the per-head [16, N] outputs.
"""

import math

import numpy as np

NHEADS = 8
DV = 128
DH = DV // NHEADS  # 16
C = 128
N = 4096
ECOLS = 512        # i-columns handled per output tile ("eighth")
NE = N // ECOLS    # 8
JW = 128           # keys per j-block
NJB = N // JW      # 32
NG = NJB // 2      # 16 j-block pairs per eighth

# exp engine per group within an eighth: 'A' ScalarE table exp,
# 'D' VectorE Schraudolph.  gi 15 must be 'A': its defer-2 stop-PV has
# to be emitted before the next eighth's gi-2 tail reads the
# denominator.
ENGMAP = ['A', 'D', 'A', 'A', 'D', 'A', 'A', 'D',
          'A', 'A', 'D', 'A', 'D', 'A', 'D', 'A']
DEFER = {'A': 2, 'D': 4}   # groups between qk and pv emission

# f32 reciprocal bit-trick: bits(1/d) ~= K - bits(d), then one
# Newton-Raphson step; max rel err 0.26% before the bf16 store.
K_RECIP = float(0x7EF30000)

A_SCHR = 128.0 / math.log(2.0)
B_SCHR = 16256.0 - 4.77

_compiled = None


def _build_program():
    import concourse.bacc as bacc
    import concourse.mybir as mybir
    import concourse.tile as tile

    f32 = mybir.dt.float32
    bf16 = mybir.dt.bfloat16
    fp16 = mybir.dt.float16
    i16 = mybir.dt.int16
    EXP = mybir.ActivationFunctionType.Exp
    LN = mybir.ActivationFunctionType.Ln
    ADD = mybir.AluOpType.add
    MULT = mybir.AluOpType.mult

    nc = bacc.Bacc("TRN2", target_bir_lowering=False, debug=False,
                   num_devices=NHEADS)

    x_d = nc.dram_tensor("x", [C, N], fp16, kind="ExternalInput")
    # w cols: 0-15 wq*scale, 16-31 wk, 32-47 wv
    w_d = nc.dram_tensor("w", [C, 48], fp16, kind="ExternalInput")
    # bias plane rows: 0-15 bq*scale (bcast), 16-31 bk+emb.  fp16: q'/k'
    # are rounded to fp16 right after the bias add anyway, so f32 bias
    # precision would be wasted; fp16 halves the startup DMA bytes.
    b_d = nc.dram_tensor("bias", [32, N], fp16, kind="ExternalInput")
    # bv tiled to [128, 32] so the v^T augment add can fold it in
    bvt_d = nc.dram_tensor("bvt", [128, 32], f32, kind="ExternalInput")
    o_d = nc.dram_tensor("out", [DH, N], f32, kind="ExternalOutput")

    with tile.TileContext(nc) as tc:
        with (
            tc.tile_pool(name="const", bufs=1) as const,
            tc.tile_pool(name="pt", bufs=8) as ptp,
            tc.tile_pool(name="o", bufs=2) as op,
            tc.tile_pool(name="st", bufs=3, space="PSUM") as stp,
            tc.tile_pool(name="acc", bufs=2, space="PSUM") as accp,
        ):
            x_s = const.tile([C, N], fp16)
            w_s = const.tile([C, 48], fp16)
            # bias in 4 column tiles so the projection add for chunk ch
            # only waits on its own quarter (tile-granular deps).
            biasf = [const.tile([32, 1024], fp16, name=f"biasf{q}")
                     for q in range(4)]
            bvt_s = const.tile([128, 32], f32)
            # K=96 contraction: the smallest K that keeps the PE HAM warm.
            # Split into per-512-column tiles so consumers only wait on
            # their own chunk (Tile tracks deps at whole-tile granularity).
            # qz rows: 0-15 q'; 16-31 k' (junk for the matmul); 32-95 zero
            qzt = [const.tile([96, 512], fp16, name=f"qzt{c}")
                   for c in range(8)]
            # kz rows: 0-15 k'; 16-95 exact zero (masks the qz junk rows)
            kzt = [const.tile([96, 512], fp16, name=f"kzt{c}")
                   for c in range(8)]
            vaugT = const.tile([128, 33 * NJB], bf16)  # [j, (v^T |0*16| 1)]
            va3 = vaugT[:].rearrange("p (c s) -> p c s", s=33)
            ones16 = const.tile([1, DH], bf16)
            zerob = const.tile([128, 1], f32)
            scratch1 = const.tile([128, 1], f32)

            # Startup DMA schedule, paced to the consumption order of the
            # first eighth (qk group 2c+t consumes kzt[2c+t] built from x
            # columns [1024c, 1024c+1024)): per round c, the two x halves
            # of chunk c go to sync+scalar in parallel while gpsimd moves
            # that round's bias quarter.  Engine-level constants (zerob
            # gates the first exp's bias) go on the otherwise-idle DVE.
            nc.vector.memset(zerob[:], 0.0)
            nc.vector.memset(ones16[:], 1.0)
            nc.sync.dma_start(w_s[:], w_d.ap())
            for q in range(4):
                qs = slice(q * 1024, (q + 1) * 1024)
                nc.sync.dma_start(x_s[0:64, qs], x_d.ap()[0:64, qs])
                nc.scalar.dma_start(x_s[64:128, qs], x_d.ap()[64:128, qs])
                nc.gpsimd.dma_start(biasf[q][:], b_d.ap()[:, qs])
            nc.gpsimd.dma_start(bvt_s[:], bvt_d.ap())
            nc.vector.memset(va3[:, :, 16:32], 0.0)
            nc.vector.memset(va3[:, :, 32:33], 1.0)
            # Warm the exp table set while DMAs run.
            nc.scalar.activation(scratch1[:], zerob[:], EXP, bias=zerob[:])

            # qkv projection: psum[0:32] = w[:, 0:32]^T @ x, + bias plane.
            # Zero-region memsets are interleaved per chunk so the first
            # chunks' adds/shifts aren't queued behind 30 memsets.
            for ch in range(8):
                cs = slice(ch * 512, (ch + 1) * 512)
                nc.vector.memset(qzt[ch][32:64, :], 0.0)
                nc.vector.memset(qzt[ch][64:96, :], 0.0)
                nc.gpsimd.memset(kzt[ch][32:64, :], 0.0)
                nc.gpsimd.memset(kzt[ch][64:96, :], 0.0)
                ps = stp.tile([32, 512], f32, tag="st")
                nc.tensor.matmul(ps[:], lhsT=w_s[:, 0:32], rhs=x_s[:, cs],
                                 start=True, stop=True)
                nc.vector.tensor_tensor(qzt[ch][0:32, :], ps[:],
                                        biasf[ch // 2][:, (ch % 2) * 512:
                                                       (ch % 2) * 512 + 512],
                                        ADD)
                # k' into the zero-padded stationary tensor (partition
                # shift; only a DMA can cross partitions).  Copying 32
                # rows brings 16 zeros from qzt's zeroed region along,
                # covering kzt rows 16:32 (engine memsets must start at a
                # 32-aligned partition, so they can't zero those rows).
                nc.gpsimd.dma_start(kzt[ch][0:2 * DH, :],
                                    qzt[ch][DH:3 * DH, :])

            # v^T psum staging; the projection MMs themselves are
            # interleaved into eighth 0 of the main loop.
            vps = stp.tile([128, 512], f32, tag="st")
            vp3 = vps[:].rearrange("p (c s) -> p c s", s=16)

            def emit_vt(jb):
                nc.tensor.matmul(vps[:, jb * 16:(jb + 1) * 16],
                                 lhsT=x_s[:, jb * JW:(jb + 1) * JW],
                                 rhs=w_s[:, 32:48],
                                 start=True, stop=True)

            bv3 = bvt_s[:].rearrange("p (c s) -> p c s", s=16)

            def emit_vt_aug(c0, c1):
                # v' = v + bv: the ones-row in vaug turns sum P bv into
                # bv * denominator, so normalize needs no post-add.
                nc.vector.tensor_tensor(va3[:, c0:c1, 0:16], vp3[:, c0:c1, :],
                                        bv3[:, 0:c1 - c0, :], ADD)

            def make_pv(pt, jbs, acc, start, stop):
                def emit():
                    for t, jb in enumerate(jbs):
                        nc.tensor.matmul(
                            acc[:],
                            lhsT=vaugT[:, 33 * jb:33 * jb + 33],
                            rhs=pt[:, 512 * t:512 * (t + 1)],
                            start=(start and t == 0),
                            stop=(stop and t == len(jbs) - 1))
                return emit

            def make_tail_a(acc):
                # 1/denom in three ~0.6us DVE ops emitted one group apart
                # (DVE's RECIPROCAL is a single 3.3us queue-blocking
                # instruction; reciprocal_approx_fast diverges on our
                # e^0..e^42 denominators; a ScalarE exp(-ln) churns
                # activation tables).  r holds (t-2)*r0 in bf16; the
                # GpSimd broadcast then fans (2-t)*r0 across rows.
                o16 = op.tile([DH, ECOLS], f32, tag="o16")
                r0 = op.tile([1, ECOLS], f32, tag="r0")
                tn = op.tile([1, ECOLS], f32, tag="tn")
                r = op.tile([1, ECOLS], bf16, tag="r")

                def emit_seed():
                    nc.vector.tensor_scalar(
                        r0[:].bitcast(mybir.dt.int32),
                        acc[32:33, :].bitcast(mybir.dt.int32),
                        -1.0, K_RECIP, MULT, ADD)

                def emit_o16():
                    # gi4 is a DVE-exp slot, so ScalarE is idle there:
                    # the copy no longer bumps an exp inside an A-run.
                    nc.scalar.copy(o16[:], acc[0:DH, :])

                def emit_nr_mul():
                    # tn = -denom*r0 so the finish computes (2-t)*r0
                    # without a separate negate.
                    nc.vector.scalar_tensor_tensor(
                        tn[:], acc[32:33, :], -1.0, r0[:], MULT, MULT)

                def emit_nr_fin():
                    # gi5 is an ACT-exp slot, so DVE is idle there: the
                    # finish no longer queues behind the gi4 exp.
                    nc.vector.scalar_tensor_tensor(
                        r[:], tn[:], 2.0, r0[:], ADD, MULT)
                return emit_seed, emit_nr_mul, emit_o16, emit_nr_fin, o16, r

            def make_tail_b(o16, r, es):
                def emit():
                    # Broadcast 1/denom across the 16 output rows on the
                    # idle GpSimd engine (frees a PE matmul + PSUM slot).
                    bc = op.tile([DH, ECOLS], bf16, tag="bc")
                    nc.gpsimd.partition_broadcast(bc[:], r[:], channels=DH)
                    ost = op.tile([DH, ECOLS], f32, tag="ost")
                    nc.vector.tensor_tensor(ost[:], o16[:], bc[:],
                                            MULT)
                    nc.sync.dma_start(o_d.ap()[:, es], ost[:])
                return emit

            from collections import deque
            pend = deque()  # (due_gidx, emit_fn)
            GROUPS = [(2 * g, 2 * g + 1) for g in range(NG)]
            pending_a = []
            pending_b = None
            for e in range(NE):
                es = slice(e * ECOLS, (e + 1) * ECOLS)
                acc = accp.tile([33, ECOLS], f32)
                for gi, jbs in enumerate(GROUPS):
                    gidx = e * NG + gi
                    if e == 0:
                        for jb in jbs:
                            emit_vt(jb)
                        emit_vt_aug(jbs[0], jbs[-1] + 1)
                    fw = 512 * len(jbs)
                    st = stp.tile([128, 1024], f32, tag="st")
                    for t, jb in enumerate(jbs):
                        kc = kzt[jb // 4][:, (jb % 4) * JW:(jb % 4 + 1) * JW]
                        nc.tensor.matmul(st[:, 512 * t:512 * (t + 1)],
                                         lhsT=kc, rhs=qzt[e][:],
                                         start=True, stop=True)
                    pt = ptp.tile([128, 1024], bf16)
                    eng = ENGMAP[gi]
                    if eng == 'A':
                        nc.scalar.activation(pt[:, 0:fw], st[:, 0:fw], EXP,
                                             bias=zerob[:])
                    else:
                        nc.vector.tensor_scalar(
                            pt[:, 0:fw].bitcast(i16), st[:, 0:fw],
                            A_SCHR, B_SCHR, MULT, ADD)
                    while pend and pend[0][0] <= gidx:
                        pend.popleft()[1]()
                    if pending_a and gi in (2, 3, 4, 5):
                        pending_a.pop(0)()
                    if pending_b is not None and gi == 6:
                        pending_b()
                        pending_b = None
                    pend.append((gidx + DEFER[eng],
                                 make_pv(pt, jbs, acc,
                                         start=(gi == 0),
                                         stop=(gi == len(GROUPS) - 1))))
                e_seed, e_mul, e_o16, e_fin, o16, r = make_tail_a(acc)
                pending_a = [e_seed, e_mul, e_o16, e_fin]
                if pending_b is not None:
                    pending_b()
                pending_b = make_tail_b(o16, r, es)
            while pend:
                pend.popleft()[1]()
            for f in pending_a:
                f()
            pending_b()

    nc.compile()
    return nc


def _get_program():
    global _compiled
    if _compiled is None:
        _compiled = _build_program()
    return _compiled


def _prepare_core_inputs(x, w_qkv, b_qkv, emb_d, emb_h, emb_w):
    x2 = np.ascontiguousarray(
        np.asarray(x, np.float32).reshape(C, N)).astype(np.float16)
    w_qkv = np.asarray(w_qkv, np.float32)
    b_qkv = np.asarray(b_qkv, np.float32)
    scale = DH ** -0.5
    emb = (np.asarray(emb_d, np.float32)
           + np.asarray(emb_h, np.float32)
           + np.asarray(emb_w, np.float32)).reshape(DH, N)
    in_maps = []
    for h in range(NHEADS):
        qc = slice(h * DH, (h + 1) * DH)
        kc = slice(DV + h * DH, DV + (h + 1) * DH)
        vc = slice(2 * DV + h * DH, 2 * DV + (h + 1) * DH)
        w = np.empty((C, 48), np.float32)
        w[:, 0:16] = w_qkv[:, qc] * scale
        w[:, 16:32] = w_qkv[:, kc]
        w[:, 32:48] = w_qkv[:, vc]
        w = w.astype(np.float16)
        bias = np.empty((32, N), np.float32)
        bias[0:16, :] = (b_qkv[qc] * scale)[:, None]
        bias[16:32, :] = b_qkv[kc][:, None] + emb
        bias = bias.astype(np.float16)
        bvt = np.ascontiguousarray(
            np.tile(b_qkv[vc].astype(np.float32), (128, 2)))
        in_maps.append({"x": x2, "w": w, "bias": bias, "bvt": bvt})
    return in_maps


def kernel(x, w_qkv, b_qkv, emb_d, emb_h, emb_w):
    from concourse.bass_utils import run_bass_kernel_spmd

    nc = _get_program()
    in_maps = _prepare_core_inputs(x, w_qkv, b_qkv, emb_d, emb_h, emb_w)
    res = run_bass_kernel_spmd(nc, in_maps, list(range(NHEADS)))
    out = np.empty((DV, N), np.float32)
    for h in range(NHEADS):
        out[h * DH:(h + 1) * DH, :] = res.results[h]["out"]
    return out.reshape(1, DV, 16, 16, 16)
